# revision 87
# baseline (speedup 1.0000x reference)
"""Trainium2 Bass kernel for nn_ExpaModel_73478300500036 (3-layer GATv2-style
GNN message passing, N=16384 nodes, E=49152 edges, D=768, H=4 heads).

Strategy (8 NeuronCores, SPMD), v3:
  - dst-shard with a host-side node permutation that (a) balances total
    in-degree across cores and (b) packs each core's 16 windows of 128
    nodes toward exactly 512 real edges -> ~3-4 edge chunks per window.
  - Per layer, hs = x @ Wsrc is computed shard-locally in two 1536-col
    halves.  Instead of AllGathering all of hs (100MB), each core
    exchanges only the deduplicated src rows its edges need via AllToAll
    (~20MB per half): host precomputes per-(core,peer) request lists;
    the device stages requested rows from its own hs half into a send
    buffer (indirect row gather), AllToAll exchanges them, and per-edge
    rows are then gathered from the receive buffer by precomputed
    positions.
  - Edge phase runs as two head-pair sweeps (cols 0:1536 / 1536:3072),
    each sweep overlapping the other half's AllToAll.  Per 128-edge
    chunk: hd[dst]+he[attr] come from one-hot matmuls (host-precomputed
    one-hot tensors; K=128 dst window / K=64 relations) - no indirect
    DMA for hd/he at all.  Self-loop chunk folds the 'mean' edge attr
    via cdtT @ he_table (never materializing he_loop).
  - p = exp(logits) (softmax shift-invariance; logits are O(10) so fp32
    exp is safe).  p is folded into the gathered hs rows so one
    unscaled one-hot (eq) serves all heads: po += eq^T @ (p*hs), and
    denominators come from pd += eq^T @ p.
  - out = po_h/den_h head mean + bias, gelu, residual; final projection
    + LayerNorm + gelu is data-parallel over nodes.

Storage dtype bf16 (activations, weights, exchanged rows), fp32 accum.
"""

import os
import sys

sys.path.insert(0, "/opt/trn_rl_repo")

# The RDH collective algorithm crashes the device in this environment;
# force mesh/ring instead (also required: AllToAll only runs on mesh).
os.environ.setdefault("NEURON_RT_DBG_RDH_CC", "0")

import ml_dtypes
import numpy as np

import concourse.bass as bass
import concourse.mybir as mybir
import concourse.tile as tile
from concourse.bass_utils import run_bass_kernel_spmd
from concourse.masks import make_identity
from concourse.vector_clock import ScopedClock

# ----------------------------------------------------------------------------
# Workaround: this container's walrus build supports at most ONE sync wait per
# instruction. (a) Tile's tail drain carries several waits -> emit them as
# separate SP EventSemaphore waits; (b) post-pass splits any remaining
# multi-wait instruction.
# ----------------------------------------------------------------------------


def _patched_drain_and_barrier(self, tick_clock, wait_clock):
    nc = self.nc
    probe = mybir.InstDrain(
        name=nc.get_next_instruction_name(), ins=[], outs=[], bass_is_fusable=False
    )
    probe.engine = mybir.EngineType.SP
    wait_clock.add_sem_waits(probe, ScopedClock({None: tick_clock.global_clock}))
    waits = []
    si = probe.sync_info
    if si is not None and si.on_wait:
        waits = list(si.on_wait)
    id2sem = {h.num: h for h in self.sems.allocated().values()}
    for w in waits:
        sem = id2sem.get(w.id)
        assert sem is not None, f"drain wait on unknown sem id {w.id}"
        nc.sync.wait_ge(sem, w.wait_value)
    nc.sync.drain()
    nc.all_engine_barrier()
    assert self.sems is not None
    popped = nc._tile_sem_poison_stack.pop()
    assert popped is self._sem_poison
    nc.clear_and_free_semaphores(list(self.sems.allocated().values()))
    nc.all_engine_barrier()


tile.TileContext._drain_and_barrier = _patched_drain_and_barrier

_split_n = [0]


def _split_multi_waits(nc):
    for f in nc.m.functions:
        for bb in f.blocks:
            insts = list(bb.instructions)
            changed = False
            new_list = []
            for inst in insts:
                si = inst.sync_info
                waits = list(si.on_wait) if (si is not None and si.on_wait) else []
                if len(waits) > 1:
                    changed = True
                    for w in waits[:-1]:
                        _split_n[0] += 1
                        ev = mybir.InstEventSemaphore(
                            name=f"evsplit-{_split_n[0]}", ins=[], outs=[]
                        )
                        ev.engine = inst.engine
                        ev.sync_info = mybir.SyncInfo(on_wait=[w], on_update=[])
                        new_list.append(ev)
                    inst.sync_info = mybir.SyncInfo(
                        on_wait=[waits[-1]],
                        on_update=list(si.on_update) if si.on_update else [],
                    )
                new_list.append(inst)
            if changed:
                bb.instructions = new_list


# ----------------------------------------------------------------------------
# Problem constants (hardcoded per spec)
# ----------------------------------------------------------------------------
NCORES = 8
N = 16384
E = 49152
D = 768
H = 4
L = 3
R = 64
HD = H * D  # 3072
FH = HD // 2  # 1536, per-sweep column half (heads 0,1 / 2,3)
NL = N // NCORES  # 2048 nodes per core
NW = NL // 128  # 16 windows per core
WSZ = 128
EPW = 512  # target real edges per window (4 full chunks)
LN_EPS = 1e-5
NEG_SLOPE = 0.2

F32 = mybir.dt.float32
BF16 = mybir.dt.bfloat16
I32 = mybir.dt.int32

AF = mybir.ActivationFunctionType
ALU = mybir.AluOpType


def build_program(cpw, nch, pc, stage_sched):
    """cpw: real chunks per window (len NW, same for all cores);
    nch = sum(cpw); pc: 128-row chunks per AllToAll block (PPAD = pc*128);
    stage_sched[w]: staging chunks (dest*pc+cc) whose source rows are all in
    windows <= w (emitted inside sweep B right after window w)."""
    PPAD = pc * 128
    nc = bass.Bass("TRN2", num_devices=NCORES)

    # ---- I/O ----
    x_in = nc.dram_tensor("x_own", [NL, D], F32, kind="ExternalInput")
    idx_in = nc.dram_tensor("idx", [128, 2 * max(nch, 1)], I32, kind="ExternalInput")
    sreq_in = nc.dram_tensor("sreq", [128, NCORES * pc], I32, kind="ExternalInput")
    ohd_in = nc.dram_tensor("ohd", [128, max(nch, 1) * 128], BF16, kind="ExternalInput")
    oha_in = nc.dram_tensor("oha", [R, max(nch, 1) * 128], BF16, kind="ExternalInput")
    eq_in = nc.dram_tensor("eq", [128, max(nch, 1) * 128], BF16, kind="ExternalInput")
    cdt_in = nc.dram_tensor("cdivT", [R, NL], BF16, kind="ExternalInput")
    ws_in = nc.dram_tensor("w_src", [L, D, HD], BF16, kind="ExternalInput")
    wd_in = nc.dram_tensor("w_dst", [L, D, HD], BF16, kind="ExternalInput")
    we_in = nc.dram_tensor("w_edge", [L, D, HD], BF16, kind="ExternalInput")
    rel_in = nc.dram_tensor("rel_emb", [R, D], BF16, kind="ExternalInput")
    att_in = nc.dram_tensor("att_rep", [L, 128, HD], BF16, kind="ExternalInput")
    bias_in = nc.dram_tensor("bias_rep", [L, 128, D], F32, kind="ExternalInput")
    pw_in = nc.dram_tensor("proj_w", [D, D], BF16, kind="ExternalInput")
    pb_in = nc.dram_tensor("pb_rep", [128, D], BF16, kind="ExternalInput")
    lng_in = nc.dram_tensor("lng_rep", [128, D], BF16, kind="ExternalInput")
    lnb_in = nc.dram_tensor("lnb_rep", [128, D], BF16, kind="ExternalInput")
    out_t = nc.dram_tensor("out", [NL, D], F32, kind="ExternalOutput")

    # ---- internal DRAM ----
    agp = [
        [nc.dram_tensor(f"ag{p}{h}", [NL, FH], BF16, kind="Internal") for h in range(2)]
        for p in range(2)
    ]
    # one exchange, halves row-interleaved per block: block j rows are
    # [PPAD half-A rows; PPAD half-B rows] (indirect gathers only support
    # full contiguous rows starting at offset 0)
    snd = nc.dram_tensor("snd", [NCORES * 2 * PPAD, FH], BF16, kind="Internal")
    rcv = nc.dram_tensor("rcv", [NCORES * 2 * PPAD, FH], BF16, kind="Internal")
    hd_own = nc.dram_tensor("hd_own", [NL, HD], BF16, kind="Internal")
    acc_d = nc.dram_tensor("acc_d", [NL, D], BF16, kind="Internal")
    xb = [nc.dram_tensor(f"xb{i}", [NL, D], F32, kind="Internal") for i in range(2)]

    with tile.TileContext(nc) as tc:
        with (
            tc.tile_pool(name="sb", bufs=1) as sp,
            tc.tile_pool(name="ps", bufs=1, space="PSUM") as pp,
        ):
            # ---- static tiles ----
            ident = sp.tile([128, 128], F32, tag="ident")
            make_identity(nc, ident[:])
            identb = sp.tile([128, 128], BF16, tag="identb")
            nc.vector.tensor_copy(out=identb[:], in_=ident[:])
            ones_b = sp.tile([128, 1], BF16, tag="ones")
            nc.vector.memset(ones_b[:], 1.0)

            idx_t = sp.tile([128, 2 * max(nch, 1)], I32, tag="idx")
            nc.sync.dma_start(out=idx_t[:], in_=idx_in[:])
            sreq_t = sp.tile([128, NCORES * pc], I32, tag="sreq")
            nc.sync.dma_start(out=sreq_t[:], in_=sreq_in[:])
            ohd_t = sp.tile([128, max(nch, 1) * 128], BF16, tag="ohd")
            nc.sync.dma_start(out=ohd_t[:], in_=ohd_in[:])
            oha_t = sp.tile([R, max(nch, 1) * 128], BF16, tag="oha")
            nc.sync.dma_start(out=oha_t[:], in_=oha_in[:])
            eq_t = sp.tile([128, max(nch, 1) * 128], BF16, tag="eq")
            nc.sync.dma_start(out=eq_t[:], in_=eq_in[:])
            cdt = sp.tile([R, NL], BF16, tag="cdt")
            nc.sync.dma_start(out=cdt[:], in_=cdt_in[:])
            rel_sb = sp.tile([R, D], BF16, tag="rel")
            nc.sync.dma_start(out=rel_sb[:], in_=rel_in[:])

            # relT [768, 64] as 6 blocks of [128, 64] side by side
            relT = sp.tile([128, 6 * R], BF16, tag="relT")
            for kt in range(6):
                pt = pp.tile([128, R], BF16, tag="pa", bufs=2)
                nc.tensor.transpose(
                    out=pt[:],
                    in_=rel_sb[:, kt * 128 : (kt + 1) * 128],
                    identity=identb[:R, :R],
                )
                nc.scalar.copy(out=relT[:, kt * R : (kt + 1) * R], in_=pt[:])

            pbb = sp.tile([128, D], BF16, tag="pbb")
            nc.sync.dma_start(out=pbb[:], in_=pb_in[:])
            lngb = sp.tile([128, D], BF16, tag="lngb")
            nc.sync.dma_start(out=lngb[:], in_=lng_in[:])
            lnbb = sp.tile([128, D], BF16, tag="lnbb")
            nc.sync.dma_start(out=lnbb[:], in_=lnb_in[:])

            def load_w_half(w_dram, l, fh):
                """Load one f-half of a [D, HD] weight into [128, kt*1536+f]."""
                wt = sp.tile([128, 6 * FH], BF16, tag="W", bufs=2)
                for kt in range(6):
                    nc.sync.dma_start(
                        out=wt[:, kt * FH : (kt + 1) * FH],
                        in_=w_dram[
                            l, kt * 128 : (kt + 1) * 128, fh * FH : (fh + 1) * FH
                        ],
                    )
                return wt

            def transpose_x(x_src):
                """x_src: DRAM [NL, D] f32 -> xT tile [128, kt*NL+n] bf16."""
                xT = sp.tile([128, 6 * NL], BF16, tag="xT")
                for m in range(NW):
                    xt = sp.tile([128, D], F32, tag="xtile", bufs=2)
                    nc.sync.dma_start(out=xt[:], in_=x_src[m * 128 : (m + 1) * 128, :])
                    for kt in range(6):
                        pt = pp.tile([128, 128], F32, tag="pa", bufs=2)
                        nc.tensor.transpose(
                            out=pt[:],
                            in_=xt[:, kt * 128 : (kt + 1) * 128],
                            identity=ident[:],
                        )
                        nc.scalar.copy(
                            out=xT[:, kt * NL + m * 128 : kt * NL + (m + 1) * 128],
                            in_=pt[:],
                        )
                return xT

            def proj_half(xT, wt, dest, dcol):
                """dest[:, dcol:dcol+FH] <- x @ W half (dest: DRAM, bf16)."""
                for m in range(NW):
                    ev = sp.tile([128, FH], BF16, tag="ev", bufs=2)
                    for s in range(3):
                        ps = pp.tile([128, 512], F32, tag="pu", bufs=3)
                        for kt in range(6):
                            nc.tensor.matmul(
                                ps[:],
                                lhsT=xT[:, kt * NL + m * 128 : kt * NL + (m + 1) * 128],
                                rhs=wt[:, kt * FH + s * 512 : kt * FH + (s + 1) * 512],
                                start=(kt == 0),
                                stop=(kt == 5),
                            )
                        nc.scalar.copy(
                            out=ev[:, s * 512 : (s + 1) * 512], in_=ps[:]
                        )
                    nc.sync.dma_start(
                        out=dest[m * 128 : (m + 1) * 128, dcol : dcol + FH], in_=ev[:]
                    )

            # ================= layers =================
            n_layers = int(os.environ.get("GAT_LAYERS", str(L)))
            do_edge = os.environ.get("GAT_EDGE", "1") == "1"
            do_cc = os.environ.get("GAT_CC", "1") == "1"

            def stage_chunk(jc, src_pair):
                """Gather request chunk jc from both ag halves into the send
                buffer rows of one exchange (halves row-interleaved)."""
                j, cc2 = divmod(jc, pc)
                for hf in range(2):
                    st = sp.tile([128, FH], BF16, tag="stg", bufs=2)
                    nc.gpsimd.indirect_dma_start(
                        out=st[:],
                        out_offset=None,
                        in_=src_pair[hf][:, :],
                        in_offset=bass.IndirectOffsetOnAxis(
                            ap=sreq_t[:, jc : jc + 1], axis=0
                        ),
                    )
                    row0 = j * 2 * PPAD + hf * PPAD + cc2 * 128
                    nc.sync.dma_start(out=snd[row0 : row0 + 128, :], in_=st[:])

            def exchange():
                if do_cc:
                    nc.gpsimd.collective_compute(
                        "AllToAll",
                        ALU.bypass,
                        ins=[snd[:]],
                        outs=[rcv[:]],
                        replica_groups=[list(range(NCORES))],
                    )

            # prologue: transpose x0, project both hs halves per window with
            # staging interleaved, then kick the exchange
            xT = transpose_x(x_in)
            wts0 = [load_w_half(ws_in, 0, hf) for hf in range(2)]
            for m in range(NW):
                for hf in range(2):
                    ev = sp.tile([128, FH], BF16, tag="ev", bufs=2)
                    for s in range(3):
                        ps = pp.tile([128, 512], F32, tag="pu", bufs=3)
                        for kt in range(6):
                            nc.tensor.matmul(
                                ps[:],
                                lhsT=xT[:, kt * NL + m * 128 : kt * NL + (m + 1) * 128],
                                rhs=wts0[hf][
                                    :, kt * FH + s * 512 : kt * FH + (s + 1) * 512
                                ],
                                start=(kt == 0),
                                stop=(kt == 5),
                            )
                        nc.scalar.copy(out=ev[:, s * 512 : (s + 1) * 512], in_=ps[:])
                    nc.sync.dma_start(
                        out=agp[0][hf][m * 128 : (m + 1) * 128, :], in_=ev[:]
                    )
                for jc in stage_sched[m]:
                    stage_chunk(jc, agp[0])
            exchange()
            x_cur = x_in

            for l in range(n_layers):
                x_next = xb[l % 2]
                ag_cur = agp[l % 2]
                ag_nxt = agp[(l + 1) % 2]

                # per-half hd projection + he table (emitted lazily inside the
                # sweep loop so half-B prep fills the wait for its exchange)
                he_tab = sp.tile([R, HD], BF16, tag="hetab")

                def prep_half(hf):
                    wt = load_w_half(wd_in, l, hf)
                    proj_half(xT, wt, hd_own, hf * FH)
                    wt = load_w_half(we_in, l, hf)
                    for s in range(3):
                        pst = pp.tile([128, 512], F32, tag="pu", bufs=3)
                        for kt in range(6):
                            nc.tensor.matmul(
                                pst[:R, :],
                                lhsT=relT[:, kt * R : (kt + 1) * R],
                                rhs=wt[:, kt * FH + s * 512 : kt * FH + (s + 1) * 512],
                                start=(kt == 0),
                                stop=(kt == 5),
                            )
                        nc.scalar.copy(
                            out=he_tab[:, hf * FH + s * 512 : hf * FH + (s + 1) * 512],
                            in_=pst[:R, :],
                        )

                att_t = sp.tile([128, HD], BF16, tag="att")
                nc.sync.dma_start(out=att_t[:], in_=att_in[l])
                bias_b = sp.tile([128, D], F32, tag="biasb")
                nc.sync.dma_start(out=bias_b[:], in_=bias_in[l])

                prep_half(0)
                prep_half(1)
                ws_next = (
                    [load_w_half(ws_in, l + 1, hf) for hf in range(2)]
                    if (do_edge and l + 1 < n_layers)
                    else None
                )

                # ---- two head-pair sweeps ----
                for half in range(2 if do_edge else 0):
                    hc = half * FH

                    def attn_logits(z, logit, p_t, p_b):
                        """logits -> p (no value folding)."""
                        for hh in range(2):
                            hg = half * 2 + hh
                            nc.vector.scalar_tensor_tensor(
                                out=z[:, hh * D : (hh + 1) * D],
                                in0=z[:, hh * D : (hh + 1) * D],
                                scalar=1.0,
                                in1=att_t[:, hg * D : (hg + 1) * D],
                                op0=ALU.mult,
                                op1=ALU.mult,
                                accum_out=logit[:, hh : hh + 1],
                            )
                        nc.scalar.activation(out=p_t[:], in_=logit[:], func=AF.Exp)
                        nc.vector.tensor_copy(out=p_b[:], in_=p_t[:])

                    def fold_p(val, p_t):
                        for hh in range(2):
                            eng = nc.vector if hh == 0 else nc.gpsimd
                            eng.tensor_scalar_mul(
                                val[:, hh * D : (hh + 1) * D],
                                in0=val[:, hh * D : (hh + 1) * D],
                                scalar1=p_t[:, hh : hh + 1],
                            )

                    pending = []

                    def emit_interleave(w2, g2):
                        """Next-layer transpose + hs projection + staging for
                        window w2 (deferred one window so it back-fills PE)."""
                        rows2 = slice(w2 * 128, (w2 + 1) * 128)
                        for kt in range(6):
                            pt = pp.tile([128, 128], F32, tag="pa", bufs=2)
                            nc.tensor.transpose(
                                out=pt[:],
                                in_=g2[:, kt * 128 : (kt + 1) * 128],
                                identity=ident[:],
                            )
                            nc.scalar.copy(
                                out=xT[
                                    :, kt * NL + w2 * 128 : kt * NL + (w2 + 1) * 128
                                ],
                                in_=pt[:],
                            )
                        if ws_next is None:
                            return
                        for hf in range(2):
                            ev = sp.tile([128, FH], BF16, tag="ev", bufs=2)
                            for s in range(3):
                                ps = pp.tile([128, 512], F32, tag="pu", bufs=3)
                                for kt in range(6):
                                    nc.tensor.matmul(
                                        ps[:],
                                        lhsT=xT[
                                            :,
                                            kt * NL + w2 * 128 : kt * NL
                                            + (w2 + 1) * 128,
                                        ],
                                        rhs=ws_next[hf][
                                            :,
                                            kt * FH + s * 512 : kt * FH + (s + 1) * 512,
                                        ],
                                        start=(kt == 0),
                                        stop=(kt == 5),
                                    )
                                nc.scalar.copy(
                                    out=ev[:, s * 512 : (s + 1) * 512], in_=ps[:]
                                )
                            nc.sync.dma_start(out=ag_nxt[hf][rows2, :], in_=ev[:])
                        for jc in stage_sched[w2]:
                            stage_chunk(jc, ag_nxt)

                    # self-loop pre-pass over all windows: needs no exchanged
                    # rows, so it fills the pipeline while the AllToAll for
                    # this half is still in flight.
                    selfp = []
                    for w in range(NW):
                        rows = slice(w * 128, (w + 1) * 128)
                        hdw0 = sp.tile([128, FH], BF16, tag="hdw", bufs=2)
                        nc.sync.dma_start(out=hdw0[:], in_=hd_own[rows, hc : hc + FH])
                        hsw0 = sp.tile([128, FH], BF16, tag="hsw", bufs=2)
                        nc.sync.dma_start(out=hsw0[:], in_=ag_cur[half][rows, :])
                        z2 = sp.tile([128, FH], BF16, tag="z", bufs=2)
                        for s in range(3):
                            sc = slice(s * 512, (s + 1) * 512)
                            pu2 = pp.tile([128, 512], F32, tag="pu", bufs=3)
                            nc.tensor.matmul(
                                pu2[:],
                                lhsT=cdt[:, rows],
                                rhs=he_tab[:, hc + s * 512 : hc + (s + 1) * 512],
                                start=True,
                                stop=False,
                            )
                            nc.tensor.matmul(
                                pu2[:], lhsT=identb[:], rhs=hsw0[:, sc],
                                start=False, stop=False,
                            )
                            nc.tensor.matmul(
                                pu2[:], lhsT=identb[:], rhs=hdw0[:, sc],
                                start=False, stop=True,
                            )
                            nc.scalar.activation(
                                out=z2[:, sc], in_=pu2[:], func=AF.Prelu,
                                alpha=NEG_SLOPE,
                            )
                        logit2 = sp.tile([128, 2], F32, tag="logit", bufs=4)
                        p2 = sp.tile([128, 2], F32, tag="pts", bufs=NW + 2)
                        p2b = sp.tile([128, 2], BF16, tag="ptbs", bufs=NW + 2)
                        attn_logits(z2, logit2, p2, p2b)
                        selfp.append((p2, p2b))

                    ci = 0
                    for w in range(NW):
                        rows = slice(w * 128, (w + 1) * 128)
                        po = pp.tile([128, FH], F32, tag="po")
                        pd = pp.tile([128, 2], F32, tag="pa", bufs=2)
                        hdw = sp.tile([128, FH], BF16, tag="hdw", bufs=2)
                        nc.sync.dma_start(out=hdw[:], in_=hd_own[rows, hc : hc + FH])
                        hsw = sp.tile([128, FH], BF16, tag="hsw", bufs=2)
                        nc.sync.dma_start(out=hsw[:], in_=ag_cur[half][rows, :])
                        nreal = cpw[w]
                        vals = []  # (value rows tile, one-hot lhsT AP, p tile)

                        p2, p2b = selfp[w]
                        fold_p(hsw, p2)
                        vals.append((hsw, identb[:], p2b))

                        for c in range(nreal):
                            cidx = ci + c
                            ccols = slice(cidx * 128, (cidx + 1) * 128)
                            hsg = sp.tile([128, FH], BF16, tag="hsg", bufs=8)
                            icol = half * max(nch, 1) + cidx
                            nc.gpsimd.indirect_dma_start(
                                out=hsg[:],
                                out_offset=None,
                                in_=(rcv if do_cc else snd)[:, :],
                                in_offset=bass.IndirectOffsetOnAxis(
                                    ap=idx_t[:, icol : icol + 1], axis=0
                                ),
                            )
                            z = sp.tile([128, FH], BF16, tag="z", bufs=2)
                            for s in range(3):
                                sc = slice(s * 512, (s + 1) * 512)
                                pu = pp.tile([128, 512], F32, tag="pu", bufs=3)
                                nc.tensor.matmul(
                                    pu[:],
                                    lhsT=ohd_t[:, ccols],
                                    rhs=hdw[:, sc],
                                    start=True,
                                    stop=False,
                                )
                                nc.tensor.matmul(
                                    pu[:],
                                    lhsT=oha_t[:, ccols],
                                    rhs=he_tab[:, hc + s * 512 : hc + (s + 1) * 512],
                                    start=False,
                                    stop=False,
                                )
                                nc.tensor.matmul(
                                    pu[:],
                                    lhsT=identb[:],
                                    rhs=hsg[:, sc],
                                    start=False,
                                    stop=True,
                                )
                                nc.scalar.activation(
                                    out=z[:, sc], in_=pu[:], func=AF.Prelu,
                                    alpha=NEG_SLOPE,
                                )
                            logit = sp.tile([128, 2], F32, tag="logit", bufs=4)
                            p_t = sp.tile([128, 2], F32, tag="pt", bufs=4)
                            p_b = sp.tile([128, 2], BF16, tag="ptb", bufs=4)
                            attn_logits(z, logit, p_t, p_b)
                            fold_p(hsg, p_t)
                            vals.append((hsg, eq_t[:, ccols], p_b))

                        # scatter: po += eq^T @ (p*hs); pd += eq^T @ p
                        nv = len(vals)
                        for s in range(3):
                            sc = slice(s * 512, (s + 1) * 512)
                            for i, (vt, lh, _) in enumerate(vals):
                                nc.tensor.matmul(
                                    po[:, sc],
                                    lhsT=lh,
                                    rhs=vt[:, sc],
                                    start=(i == 0),
                                    stop=(i == nv - 1),
                                )
                        for i, (vt, lh, pt) in enumerate(vals):
                            nc.tensor.matmul(
                                pd[:, 0:2],
                                lhsT=lh,
                                rhs=pt[:],
                                start=(i == 0),
                                stop=(i == nv - 1),
                            )

                        # finalize window for this head pair
                        rden = sp.tile([128, 2], F32, tag="rden", bufs=2)
                        nc.vector.reciprocal(out=rden[:], in_=pd[:])
                        nc.vector.tensor_scalar_mul(
                            rden[:], in0=rden[:], scalar1=1.0 / H
                        )
                        if half == 0:
                            acct = sp.tile([128, D], BF16, tag="acct", bufs=2)
                            nc.scalar.activation(
                                out=acct[:],
                                in_=po[:, 0:D],
                                func=AF.Copy,
                                scale=rden[:, 0:1],
                            )
                            nc.vector.scalar_tensor_tensor(
                                out=acct[:],
                                in0=po[:, D : 2 * D],
                                scalar=rden[:, 1:2],
                                in1=acct[:],
                                op0=ALU.mult,
                                op1=ALU.add,
                            )
                            nc.sync.dma_start(out=acc_d[rows, :], in_=acct[:])
                        else:
                            acct = sp.tile([128, D], BF16, tag="acct", bufs=2)
                            nc.sync.dma_start(out=acct[:], in_=acc_d[rows, :])
                            fin = sp.tile([128, D], F32, tag="fin", bufs=2)
                            nc.vector.scalar_tensor_tensor(
                                out=fin[:],
                                in0=po[:, 0:D],
                                scalar=rden[:, 0:1],
                                in1=acct[:],
                                op0=ALU.mult,
                                op1=ALU.add,
                            )
                            nc.vector.scalar_tensor_tensor(
                                out=fin[:],
                                in0=po[:, D : 2 * D],
                                scalar=rden[:, 1:2],
                                in1=fin[:],
                                op0=ALU.mult,
                                op1=ALU.add,
                            )
                            nc.vector.tensor_add(out=fin[:], in0=fin[:], in1=bias_b[:])
                            g_t = sp.tile([128, D], F32, tag="fin2", bufs=3)
                            nc.scalar.activation(
                                out=g_t[:], in_=fin[:], func=AF.Gelu_apprx_tanh
                            )
                            xc = sp.tile([128, D], F32, tag="fin", bufs=2)
                            nc.sync.dma_start(out=xc[:], in_=x_cur[rows, :])
                            nc.vector.tensor_add(out=g_t[:], in0=g_t[:], in1=xc[:])
                            nc.sync.dma_start(out=x_next[rows, :], in_=g_t[:])
                            pending.append((w, g_t))
                        ci += nreal
                        if len(pending) > 1:
                            emit_interleave(*pending.pop(0))
                    while pending:
                        emit_interleave(*pending.pop(0))

                if do_edge:
                    if ws_next is not None:
                        exchange()
                    x_cur = x_next

            # ================= projection + LayerNorm + gelu =================
            if os.environ.get("GAT_PROJ", "1") != "1":
                for m in range(NW):
                    rows = slice(m * 128, (m + 1) * 128)
                    ct = sp.tile([128, D], F32, tag="fin", bufs=2)
                    nc.sync.dma_start(out=ct[:], in_=x_cur[rows, :])
                    nc.sync.dma_start(out=out_t[rows, :], in_=ct[:])
            else:
                # proj weights, laid out [128, kt*768+f]; shares tag "W"
                pwt = sp.tile([128, 6 * D], BF16, tag="W", bufs=2)
                for kt in range(6):
                    nc.sync.dma_start(
                        out=pwt[:, kt * D : (kt + 1) * D],
                        in_=pw_in[kt * 128 : (kt + 1) * 128, :],
                    )
                # xT already holds the final x (interleaved transposes)
                for m in range(NW):
                    rows = slice(m * 128, (m + 1) * 128)
                    ps = pp.tile([128, D], F32, tag="po")
                    for kt in range(6):
                        for a, b in ((0, 512), (512, 768)):
                            nc.tensor.matmul(
                                ps[:, a:b],
                                lhsT=xT[:, kt * NL + m * 128 : kt * NL + (m + 1) * 128],
                                rhs=pwt[:, kt * D + a : kt * D + b],
                                start=(kt == 0),
                                stop=(kt == 5),
                            )
                    y0 = sp.tile([128, D], F32, tag="fin", bufs=2)
                    nc.vector.tensor_add(out=y0[:], in0=ps[:], in1=pbb[:])
                    mu = sp.tile([128, 1], F32, tag="stats", bufs=4)
                    nc.vector.tensor_reduce(
                        out=mu[:], in_=y0[:], axis=mybir.AxisListType.X, op=ALU.add
                    )
                    nc.vector.tensor_scalar_mul(mu[:], in0=mu[:], scalar1=1.0 / D)
                    xc2 = sp.tile([128, D], F32, tag="fin2", bufs=3)
                    nc.vector.tensor_scalar_sub(xc2[:], in0=y0[:], scalar1=mu[:])
                    var = sp.tile([128, 1], F32, tag="stats", bufs=4)
                    nc.vector.scalar_tensor_tensor(
                        out=y0[:],
                        in0=xc2[:],
                        scalar=1.0,
                        in1=xc2[:],
                        op0=ALU.mult,
                        op1=ALU.mult,
                        accum_out=var[:],
                    )
                    nc.vector.tensor_scalar(
                        var[:],
                        in0=var[:],
                        scalar1=1.0 / D,
                        scalar2=LN_EPS,
                        op0=ALU.mult,
                        op1=ALU.add,
                    )
                    sd = sp.tile([128, 1], F32, tag="stats", bufs=4)
                    nc.scalar.activation(out=sd[:], in_=var[:], func=AF.Sqrt)
                    rstd = sp.tile([128, 1], F32, tag="stats", bufs=4)
                    nc.vector.reciprocal(out=rstd[:], in_=sd[:])
                    nc.vector.tensor_scalar_mul(y0[:], in0=xc2[:], scalar1=rstd[:])
                    nc.vector.tensor_mul(out=y0[:], in0=y0[:], in1=lngb[:])
                    nc.vector.tensor_add(out=y0[:], in0=y0[:], in1=lnbb[:])
                    og = sp.tile([128, D], F32, tag="fin", bufs=2)
                    nc.scalar.activation(out=og[:], in_=y0[:], func=AF.Gelu_apprx_tanh)
                    nc.sync.dma_start(out=out_t[rows, :], in_=og[:])

    _split_multi_waits(nc)
    return nc


# ----------------------------------------------------------------------------
# Host side
# ----------------------------------------------------------------------------


def _preprocess(edge_index, edge_attr):
    src = np.asarray(edge_index[0], dtype=np.int64)
    dst = np.asarray(edge_index[1], dtype=np.int64)
    attr = np.asarray(edge_attr, dtype=np.int64)
    deg = np.bincount(dst, minlength=N).astype(np.int64)

    # ---- node permutation: balance cores by degree, pack windows to EPW ----
    order = np.argsort(-deg, kind="stable")
    core_of = np.empty(N, np.int64)
    cload = np.zeros(NCORES, np.int64)
    ccnt = np.zeros(NCORES, np.int64)
    for n in order:
        k = int(np.argmin(np.where(ccnt < NL, cload, np.iinfo(np.int64).max)))
        core_of[n] = k
        cload[k] += deg[n]
        ccnt[k] += 1

    new_id = np.empty(N, np.int64)
    for k in range(NCORES):
        nodes = np.where(core_of == k)[0]
        nodes = nodes[np.argsort(-deg[nodes], kind="stable")]
        wload = np.zeros(NW, np.int64)
        wcnt = np.zeros(NW, np.int64)
        wassign = np.empty(len(nodes), np.int64)
        for i, n in enumerate(nodes):
            d = deg[n]
            open_w = wcnt < WSZ
            fits = open_w & (wload + d <= EPW)
            if fits.any():
                # best-fit: fullest window that still fits
                j = int(np.argmax(np.where(fits, wload, -1)))
            else:
                j = int(np.argmin(np.where(open_w, wload, np.iinfo(np.int64).max)))
            wassign[i] = j
            wload[j] += d
            wcnt[j] += 1
        # swap-rebalance: no window should exceed EPW (each extra 128 costs a
        # whole padded chunk on every core via the max-over-cores cpw)
        degs = deg[nodes]
        for _ in range(200):
            wo = int(np.argmax(wload))
            if wload[wo] <= EPW:
                break
            done = False
            cand_a = np.where(wassign == wo)[0]
            cand_a = cand_a[np.argsort(-degs[cand_a], kind="stable")]
            for wu in np.argsort(wload, kind="stable"):
                if done or wload[wu] >= wload[wo]:
                    break
                cand_b = np.where(wassign == wu)[0]
                b = int(cand_b[np.argmin(degs[cand_b])])
                for a in cand_a:
                    da, db = int(degs[a]), int(degs[b])
                    if da > db and wload[wu] + da - db <= EPW:
                        wassign[a], wassign[b] = wu, wo
                        wload[wo] += db - da
                        wload[wu] += da - db
                        done = True
                        break
            if not done:
                break
        # relabel windows by descending load so full windows align across cores
        worder = np.argsort(-wload, kind="stable")
        wrank = np.empty(NW, np.int64)
        wrank[worder] = np.arange(NW)
        slot = np.zeros(NW, np.int64)
        for i, n in enumerate(nodes):
            j = wrank[wassign[i]]
            new_id[n] = k * NL + j * 128 + slot[j]
            slot[j] += 1

    perm = np.empty(N, np.int64)  # new -> old
    perm[new_id] = np.arange(N)

    srcN = new_id[src]
    dstN = new_id[dst]
    k_e = dstN // NL
    w_e = (dstN % NL) // 128
    slot_e = dstN % 128

    # real chunks per window index (max over cores)
    loads = np.zeros((NCORES, NW), np.int64)
    np.add.at(loads, (k_e, w_e), 1)
    cpw = [int(x) for x in np.ceil(loads.max(axis=0) / 128).astype(np.int64)]
    nch = int(sum(cpw))
    cstart = np.concatenate([[0], np.cumsum(cpw)])

    # ---- A2A request lists ----
    own_e = srcN // NL  # owner core of each edge's src
    reqs = [[None] * NCORES for _ in range(NCORES)]  # reqs[j][o]
    pmax = 1
    for j in range(NCORES):
        em = k_e == j
        for o in range(NCORES):
            rows = np.unique(srcN[em & (own_e == o)] % NL)
            reqs[j][o] = rows
            pmax = max(pmax, len(rows))
    pc = -(-pmax // 128)
    ppad = pc * 128

    sreq_all = []
    for k in range(NCORES):
        arr = np.zeros((128, NCORES * pc), np.int32)
        for j in range(NCORES):
            r = reqs[j][k]
            col = np.zeros(ppad, np.int32)
            col[: len(r)] = r
            arr[:, j * pc : (j + 1) * pc] = col.reshape(pc, 128).T
        sreq_all.append(np.ascontiguousarray(arr))

    # earliest sweep-B window after which each staging chunk can be gathered
    # (requests are sorted, so chunk cc covers a contiguous row range)
    wmax = np.zeros(NCORES * pc, np.int64)
    for j in range(NCORES):
        for k in range(NCORES):
            r = reqs[j][k]
            for cc in range(pc):
                seg = r[cc * 128 : (cc + 1) * 128]
                if len(seg):
                    wmax[j * pc + cc] = max(wmax[j * pc + cc], int(seg.max()) // 128)
    stage_sched = [[] for _ in range(NW)]
    for jc in range(NCORES * pc):
        stage_sched[int(wmax[jc])].append(jc)

    # ---- per-core chunk data ----
    # idx columns [0:nch] = half-A receive rows, [nch:2nch] = half-B
    # (halves row-interleaved per source block: A at +0, B at +ppad)
    idx_all, ohd_all, oha_all, eq_all = [], [], [], []
    for j in range(NCORES):
        idx = np.zeros((128, 2 * nch), np.int32)
        ohd = np.zeros((128, nch * 128), np.float32)
        oha = np.zeros((R, nch * 128), np.float32)
        eqm = np.zeros((128, nch * 128), np.float32)
        em = np.where(k_e == j)[0]
        eo = em[np.argsort(w_e[em], kind="stable")]
        wcnt2 = np.bincount(w_e[em], minlength=NW)
        wst = np.concatenate([[0], np.cumsum(wcnt2)])
        for w in range(NW):
            es = eo[wst[w] : wst[w + 1]]
            base = cstart[w] * 128
            for i, e in enumerate(es):
                c, r2 = divmod(i, 128)
                col = base + c * 128 + r2
                o = own_e[e]
                pos = int(np.searchsorted(reqs[j][o], srcN[e] % NL))
                idx[r2, cstart[w] + c] = o * 2 * ppad + pos
                idx[r2, nch + cstart[w] + c] = o * 2 * ppad + ppad + pos
                ohd[slot_e[e], col] = 1.0
                oha[attr[e], col] = 1.0
                eqm[r2, base + c * 128 + slot_e[e]] = 1.0
        idx_all.append(idx)
        ohd_all.append(ohd.astype(ml_dtypes.bfloat16))
        oha_all.append(oha.astype(ml_dtypes.bfloat16))
        eq_all.append(eqm.astype(ml_dtypes.bfloat16))

    # ---- Cdiv (self-loop mean edge attr), new node order ----
    C = np.zeros((N, R), np.float32)
    np.add.at(C, (dstN, attr), 1.0)
    degN = np.bincount(dstN, minlength=N).astype(np.float32)
    Cdiv = C / np.maximum(degN, 1.0)[:, None]

    return (
        cpw, nch, pc, stage_sched, perm, new_id,
        sreq_all, idx_all, ohd_all, oha_all, eq_all, Cdiv,
    )


_cache = {}
_prep_cache = {}
LAST_RESULTS = None
LAST_EXEC_NS = None


def prepare(**inputs):
    x = np.asarray(inputs["x"], np.float32)
    rel_emb = np.asarray(inputs["rel_emb"], np.float32)
    w_src = np.asarray(inputs["w_src"], np.float32)
    w_dst = np.asarray(inputs["w_dst"], np.float32)
    w_edge = np.asarray(inputs["w_edge"], np.float32)
    att = np.asarray(inputs["att"], np.float32)
    bias = np.asarray(inputs["bias"], np.float32)
    proj_w = np.asarray(inputs["proj_w"], np.float32)
    proj_b = np.asarray(inputs["proj_b"], np.float32)
    ln_g = np.asarray(inputs["ln_g"], np.float32)
    ln_b = np.asarray(inputs["ln_b"], np.float32)
    edge_index = np.asarray(inputs["edge_index"], np.int32)
    edge_attr = np.asarray(inputs["edge_attr"], np.int32)

    ekey = (edge_index.tobytes(), edge_attr.tobytes())
    ck = hash(ekey)
    if ck not in _prep_cache:
        _prep_cache[ck] = _preprocess(edge_index, edge_attr)
    (
        cpw, nch, pc, stage_sched, perm, new_id,
        sreq_all, idx_all, ohd_all, oha_all, eq_all, Cdiv,
    ) = _prep_cache[ck]

    key = (tuple(cpw), nch, pc, tuple(tuple(s) for s in stage_sched))
    if key not in _cache:
        _cache[key] = build_program(cpw, nch, pc, stage_sched)
    nc = _cache[key]

    bf = lambda a: np.ascontiguousarray(a).astype(ml_dtypes.bfloat16)
    ws_b = bf(w_src.reshape(L, D, HD))
    wd_b = bf(w_dst.reshape(L, D, HD))
    we_b = bf(w_edge.reshape(L, D, HD))
    rel_b = bf(rel_emb)
    att_rep = bf(np.broadcast_to(att.reshape(L, 1, HD), (L, 128, HD)))
    bias_rep = np.ascontiguousarray(
        np.broadcast_to(bias.reshape(L, 1, D), (L, 128, D)), dtype=np.float32
    )
    pw_b = bf(proj_w)
    pb_rep = bf(np.broadcast_to(proj_b, (128, D)))
    lng_rep = bf(np.broadcast_to(ln_g, (128, D)))
    lnb_rep = bf(np.broadcast_to(ln_b, (128, D)))

    in_maps = []
    for k in range(NCORES):
        rows = perm[k * NL : (k + 1) * NL]
        in_maps.append(
            {
                "x_own": np.ascontiguousarray(x[rows]),
                "idx": idx_all[k],
                "sreq": sreq_all[k],
                "ohd": ohd_all[k],
                "oha": oha_all[k],
                "eq": eq_all[k],
                "cdivT": bf(Cdiv[k * NL : (k + 1) * NL].T),
                "w_src": ws_b,
                "w_dst": wd_b,
                "w_edge": we_b,
                "rel_emb": rel_b,
                "att_rep": att_rep,
                "bias_rep": bias_rep,
                "proj_w": pw_b,
                "pb_rep": pb_rep,
                "lng_rep": lng_rep,
                "lnb_rep": lnb_rep,
            }
        )
    return nc, in_maps, new_id


def kernel(**inputs):
    nc, in_maps, new_id = prepare(**inputs)
    trace = os.environ.get("GAT_TRACE", "0") == "1"
    res = run_bass_kernel_spmd(nc, in_maps, core_ids=list(range(NCORES)), trace=trace)
    global LAST_RESULTS, LAST_EXEC_NS
    LAST_RESULTS = res.results
    LAST_EXEC_NS = res.exec_time_ns
    arr = np.concatenate([res.results[k]["out"] for k in range(NCORES)], axis=0)
    return arr[new_id].astype(np.float32)


# revision 88
# speedup vs baseline: 1.0055x; 1.0055x over previous
"""Trainium2 Bass kernel for nn_ExpaModel_73478300500036 (3-layer GATv2-style
GNN message passing, N=16384 nodes, E=49152 edges, D=768, H=4 heads).

Strategy (8 NeuronCores, SPMD), v3:
  - dst-shard with a host-side node permutation that (a) balances total
    in-degree across cores and (b) packs each core's 16 windows of 128
    nodes toward exactly 512 real edges -> ~3-4 edge chunks per window.
  - Per layer, hs = x @ Wsrc is computed shard-locally in two 1536-col
    halves.  Instead of AllGathering all of hs (100MB), each core
    exchanges only the deduplicated src rows its edges need via AllToAll
    (~20MB per half): host precomputes per-(core,peer) request lists;
    the device stages requested rows from its own hs half into a send
    buffer (indirect row gather), AllToAll exchanges them, and per-edge
    rows are then gathered from the receive buffer by precomputed
    positions.
  - Edge phase runs as two head-pair sweeps (cols 0:1536 / 1536:3072),
    each sweep overlapping the other half's AllToAll.  Per 128-edge
    chunk: hd[dst]+he[attr] come from one-hot matmuls (host-precomputed
    one-hot tensors; K=128 dst window / K=64 relations) - no indirect
    DMA for hd/he at all.  Self-loop chunk folds the 'mean' edge attr
    via cdtT @ he_table (never materializing he_loop).
  - p = exp(logits) (softmax shift-invariance; logits are O(10) so fp32
    exp is safe).  p is folded into the gathered hs rows so one
    unscaled one-hot (eq) serves all heads: po += eq^T @ (p*hs), and
    denominators come from pd += eq^T @ p.
  - out = po_h/den_h head mean + bias, gelu, residual; final projection
    + LayerNorm + gelu is data-parallel over nodes.

Storage dtype bf16 (activations, weights, exchanged rows), fp32 accum.
"""

import os
import sys

sys.path.insert(0, "/opt/trn_rl_repo")

# The RDH collective algorithm crashes the device in this environment;
# force mesh/ring instead (also required: AllToAll only runs on mesh).
os.environ.setdefault("NEURON_RT_DBG_RDH_CC", "0")

import ml_dtypes
import numpy as np

import concourse.bass as bass
import concourse.mybir as mybir
import concourse.tile as tile
from concourse.bass_utils import run_bass_kernel_spmd
from concourse.masks import make_identity
from concourse.vector_clock import ScopedClock

# ----------------------------------------------------------------------------
# Workaround: this container's walrus build supports at most ONE sync wait per
# instruction. (a) Tile's tail drain carries several waits -> emit them as
# separate SP EventSemaphore waits; (b) post-pass splits any remaining
# multi-wait instruction.
# ----------------------------------------------------------------------------


def _patched_drain_and_barrier(self, tick_clock, wait_clock):
    nc = self.nc
    probe = mybir.InstDrain(
        name=nc.get_next_instruction_name(), ins=[], outs=[], bass_is_fusable=False
    )
    probe.engine = mybir.EngineType.SP
    wait_clock.add_sem_waits(probe, ScopedClock({None: tick_clock.global_clock}))
    waits = []
    si = probe.sync_info
    if si is not None and si.on_wait:
        waits = list(si.on_wait)
    id2sem = {h.num: h for h in self.sems.allocated().values()}
    for w in waits:
        sem = id2sem.get(w.id)
        assert sem is not None, f"drain wait on unknown sem id {w.id}"
        nc.sync.wait_ge(sem, w.wait_value)
    nc.sync.drain()
    nc.all_engine_barrier()
    assert self.sems is not None
    popped = nc._tile_sem_poison_stack.pop()
    assert popped is self._sem_poison
    nc.clear_and_free_semaphores(list(self.sems.allocated().values()))
    nc.all_engine_barrier()


tile.TileContext._drain_and_barrier = _patched_drain_and_barrier

_split_n = [0]


def _split_multi_waits(nc):
    for f in nc.m.functions:
        for bb in f.blocks:
            insts = list(bb.instructions)
            changed = False
            new_list = []
            for inst in insts:
                si = inst.sync_info
                waits = list(si.on_wait) if (si is not None and si.on_wait) else []
                if len(waits) > 1:
                    changed = True
                    for w in waits[:-1]:
                        _split_n[0] += 1
                        ev = mybir.InstEventSemaphore(
                            name=f"evsplit-{_split_n[0]}", ins=[], outs=[]
                        )
                        ev.engine = inst.engine
                        ev.sync_info = mybir.SyncInfo(on_wait=[w], on_update=[])
                        new_list.append(ev)
                    inst.sync_info = mybir.SyncInfo(
                        on_wait=[waits[-1]],
                        on_update=list(si.on_update) if si.on_update else [],
                    )
                new_list.append(inst)
            if changed:
                bb.instructions = new_list


# ----------------------------------------------------------------------------
# Problem constants (hardcoded per spec)
# ----------------------------------------------------------------------------
NCORES = 8
N = 16384
E = 49152
D = 768
H = 4
L = 3
R = 64
HD = H * D  # 3072
FH = HD // 2  # 1536, per-sweep column half (heads 0,1 / 2,3)
NL = N // NCORES  # 2048 nodes per core
NW = NL // 128  # 16 windows per core
WSZ = 128
EPW = 512  # target real edges per window (4 full chunks)
LN_EPS = 1e-5
NEG_SLOPE = 0.2

F32 = mybir.dt.float32
BF16 = mybir.dt.bfloat16
I32 = mybir.dt.int32

AF = mybir.ActivationFunctionType
ALU = mybir.AluOpType


def build_program(cpw, nch, pc, stage_sched):
    """cpw: real chunks per window (len NW, same for all cores);
    nch = sum(cpw); pc: 128-row chunks per AllToAll block (PPAD = pc*128);
    stage_sched[w]: staging chunks (dest*pc+cc) whose source rows are all in
    windows <= w (emitted inside sweep B right after window w)."""
    PPAD = pc * 128
    nc = bass.Bass("TRN2", num_devices=NCORES)

    # ---- I/O ----
    x_in = nc.dram_tensor("x_own", [NL, D], F32, kind="ExternalInput")
    idx_in = nc.dram_tensor("idx", [128, 2 * max(nch, 1)], I32, kind="ExternalInput")
    sreq_in = nc.dram_tensor("sreq", [128, NCORES * pc], I32, kind="ExternalInput")
    ohd_in = nc.dram_tensor("ohd", [128, max(nch, 1) * 128], BF16, kind="ExternalInput")
    oha_in = nc.dram_tensor("oha", [R, max(nch, 1) * 128], BF16, kind="ExternalInput")
    eq_in = nc.dram_tensor("eq", [128, max(nch, 1) * 128], BF16, kind="ExternalInput")
    cdt_in = nc.dram_tensor("cdivT", [R, NL], BF16, kind="ExternalInput")
    ws_in = nc.dram_tensor("w_src", [L, D, HD], BF16, kind="ExternalInput")
    wd_in = nc.dram_tensor("w_dst", [L, D, HD], BF16, kind="ExternalInput")
    we_in = nc.dram_tensor("w_edge", [L, D, HD], BF16, kind="ExternalInput")
    rel_in = nc.dram_tensor("rel_emb", [R, D], BF16, kind="ExternalInput")
    att_in = nc.dram_tensor("att_rep", [L, 128, HD], BF16, kind="ExternalInput")
    bias_in = nc.dram_tensor("bias_rep", [L, 128, D], F32, kind="ExternalInput")
    pw_in = nc.dram_tensor("proj_w", [D, D], BF16, kind="ExternalInput")
    pb_in = nc.dram_tensor("pb_rep", [128, D], BF16, kind="ExternalInput")
    lng_in = nc.dram_tensor("lng_rep", [128, D], BF16, kind="ExternalInput")
    lnb_in = nc.dram_tensor("lnb_rep", [128, D], BF16, kind="ExternalInput")
    out_t = nc.dram_tensor("out", [NL, D], F32, kind="ExternalOutput")

    # ---- internal DRAM ----
    agp = [
        [nc.dram_tensor(f"ag{p}{h}", [NL, FH], BF16, kind="Internal") for h in range(2)]
        for p in range(2)
    ]
    # one exchange, halves row-interleaved per block: block j rows are
    # [PPAD half-A rows; PPAD half-B rows] (indirect gathers only support
    # full contiguous rows starting at offset 0)
    snd = nc.dram_tensor("snd", [NCORES * 2 * PPAD, FH], BF16, kind="Internal")
    rcv = nc.dram_tensor("rcv", [NCORES * 2 * PPAD, FH], BF16, kind="Internal")
    hd_own = nc.dram_tensor("hd_own", [NL, HD], BF16, kind="Internal")
    acc_d = nc.dram_tensor("acc_d", [NL, D], BF16, kind="Internal")
    xb = [nc.dram_tensor(f"xb{i}", [NL, D], F32, kind="Internal") for i in range(2)]

    with tile.TileContext(nc) as tc:
        with (
            tc.tile_pool(name="sb", bufs=1) as sp,
            tc.tile_pool(name="ps", bufs=1, space="PSUM") as pp,
        ):
            # ---- static tiles ----
            ident = sp.tile([128, 128], F32, tag="ident")
            make_identity(nc, ident[:])
            identb = sp.tile([128, 128], BF16, tag="identb")
            nc.vector.tensor_copy(out=identb[:], in_=ident[:])
            ones_b = sp.tile([128, 1], BF16, tag="ones")
            nc.vector.memset(ones_b[:], 1.0)

            idx_t = sp.tile([128, 2 * max(nch, 1)], I32, tag="idx")
            nc.sync.dma_start(out=idx_t[:], in_=idx_in[:])
            sreq_t = sp.tile([128, NCORES * pc], I32, tag="sreq")
            nc.sync.dma_start(out=sreq_t[:], in_=sreq_in[:])
            ohd_t = sp.tile([128, max(nch, 1) * 128], BF16, tag="ohd")
            nc.sync.dma_start(out=ohd_t[:], in_=ohd_in[:])
            oha_t = sp.tile([R, max(nch, 1) * 128], BF16, tag="oha")
            nc.sync.dma_start(out=oha_t[:], in_=oha_in[:])
            eq_t = sp.tile([128, max(nch, 1) * 128], BF16, tag="eq")
            nc.sync.dma_start(out=eq_t[:], in_=eq_in[:])
            cdt = sp.tile([R, NL], BF16, tag="cdt")
            nc.sync.dma_start(out=cdt[:], in_=cdt_in[:])
            rel_sb = sp.tile([R, D], BF16, tag="rel")
            nc.sync.dma_start(out=rel_sb[:], in_=rel_in[:])

            # relT [768, 64] as 6 blocks of [128, 64] side by side
            relT = sp.tile([128, 6 * R], BF16, tag="relT")
            for kt in range(6):
                pt = pp.tile([128, R], BF16, tag="pa", bufs=2)
                nc.tensor.transpose(
                    out=pt[:],
                    in_=rel_sb[:, kt * 128 : (kt + 1) * 128],
                    identity=identb[:R, :R],
                )
                nc.scalar.copy(out=relT[:, kt * R : (kt + 1) * R], in_=pt[:])

            pbb = sp.tile([128, D], BF16, tag="pbb")
            nc.sync.dma_start(out=pbb[:], in_=pb_in[:])
            lngb = sp.tile([128, D], BF16, tag="lngb")
            nc.sync.dma_start(out=lngb[:], in_=lng_in[:])
            lnbb = sp.tile([128, D], BF16, tag="lnbb")
            nc.sync.dma_start(out=lnbb[:], in_=lnb_in[:])

            def load_w_half(w_dram, l, fh):
                """Load one f-half of a [D, HD] weight into [128, kt*1536+f]."""
                wt = sp.tile([128, 6 * FH], BF16, tag="W", bufs=2)
                for kt in range(6):
                    nc.sync.dma_start(
                        out=wt[:, kt * FH : (kt + 1) * FH],
                        in_=w_dram[
                            l, kt * 128 : (kt + 1) * 128, fh * FH : (fh + 1) * FH
                        ],
                    )
                return wt

            def transpose_x(x_src):
                """x_src: DRAM [NL, D] f32 -> xT tile [128, kt*NL+n] bf16."""
                xT = sp.tile([128, 6 * NL], BF16, tag="xT")
                for m in range(NW):
                    xt = sp.tile([128, D], F32, tag="xtile", bufs=1)
                    nc.sync.dma_start(out=xt[:], in_=x_src[m * 128 : (m + 1) * 128, :])
                    for kt in range(6):
                        pt = pp.tile([128, 128], F32, tag="pa", bufs=2)
                        nc.tensor.transpose(
                            out=pt[:],
                            in_=xt[:, kt * 128 : (kt + 1) * 128],
                            identity=ident[:],
                        )
                        nc.scalar.copy(
                            out=xT[:, kt * NL + m * 128 : kt * NL + (m + 1) * 128],
                            in_=pt[:],
                        )
                return xT

            def proj_half(xT, wt, dest, dcol):
                """dest[:, dcol:dcol+FH] <- x @ W half (dest: DRAM, bf16)."""
                for m in range(NW):
                    ev = sp.tile([128, FH], BF16, tag="ev", bufs=2)
                    for s in range(3):
                        ps = pp.tile([128, 512], F32, tag="pu", bufs=3)
                        for kt in range(6):
                            nc.tensor.matmul(
                                ps[:],
                                lhsT=xT[:, kt * NL + m * 128 : kt * NL + (m + 1) * 128],
                                rhs=wt[:, kt * FH + s * 512 : kt * FH + (s + 1) * 512],
                                start=(kt == 0),
                                stop=(kt == 5),
                            )
                        nc.scalar.copy(
                            out=ev[:, s * 512 : (s + 1) * 512], in_=ps[:]
                        )
                    nc.sync.dma_start(
                        out=dest[m * 128 : (m + 1) * 128, dcol : dcol + FH], in_=ev[:]
                    )

            # ================= layers =================
            n_layers = int(os.environ.get("GAT_LAYERS", str(L)))
            do_edge = os.environ.get("GAT_EDGE", "1") == "1"
            do_cc = os.environ.get("GAT_CC", "1") == "1"

            def stage_chunk(jc, src_pair):
                """Gather request chunk jc from both ag halves into the send
                buffer rows of one exchange (halves row-interleaved)."""
                j, cc2 = divmod(jc, pc)
                for hf in range(2):
                    st = sp.tile([128, FH], BF16, tag="stg", bufs=2)
                    nc.gpsimd.indirect_dma_start(
                        out=st[:],
                        out_offset=None,
                        in_=src_pair[hf][:, :],
                        in_offset=bass.IndirectOffsetOnAxis(
                            ap=sreq_t[:, jc : jc + 1], axis=0
                        ),
                    )
                    row0 = j * 2 * PPAD + hf * PPAD + cc2 * 128
                    nc.sync.dma_start(out=snd[row0 : row0 + 128, :], in_=st[:])

            def exchange():
                if do_cc:
                    nc.gpsimd.collective_compute(
                        "AllToAll",
                        ALU.bypass,
                        ins=[snd[:]],
                        outs=[rcv[:]],
                        replica_groups=[list(range(NCORES))],
                    )

            # prologue: transpose x0, project both hs halves per window with
            # staging interleaved, then kick the exchange
            xT = transpose_x(x_in)
            wts0 = [load_w_half(ws_in, 0, hf) for hf in range(2)]
            for m in range(NW):
                for hf in range(2):
                    ev = sp.tile([128, FH], BF16, tag="ev", bufs=2)
                    for s in range(3):
                        ps = pp.tile([128, 512], F32, tag="pu", bufs=3)
                        for kt in range(6):
                            nc.tensor.matmul(
                                ps[:],
                                lhsT=xT[:, kt * NL + m * 128 : kt * NL + (m + 1) * 128],
                                rhs=wts0[hf][
                                    :, kt * FH + s * 512 : kt * FH + (s + 1) * 512
                                ],
                                start=(kt == 0),
                                stop=(kt == 5),
                            )
                        nc.scalar.copy(out=ev[:, s * 512 : (s + 1) * 512], in_=ps[:])
                    nc.sync.dma_start(
                        out=agp[0][hf][m * 128 : (m + 1) * 128, :], in_=ev[:]
                    )
                for jc in stage_sched[m]:
                    stage_chunk(jc, agp[0])
            exchange()
            x_cur = x_in

            for l in range(n_layers):
                x_next = xb[l % 2]
                ag_cur = agp[l % 2]
                ag_nxt = agp[(l + 1) % 2]

                # per-half hd projection + he table (emitted lazily inside the
                # sweep loop so half-B prep fills the wait for its exchange)
                he_tab = sp.tile([R, HD], BF16, tag="hetab")

                def prep_half(hf):
                    wt = load_w_half(wd_in, l, hf)
                    proj_half(xT, wt, hd_own, hf * FH)
                    wt = load_w_half(we_in, l, hf)
                    for s in range(3):
                        pst = pp.tile([128, 512], F32, tag="pu", bufs=3)
                        for kt in range(6):
                            nc.tensor.matmul(
                                pst[:R, :],
                                lhsT=relT[:, kt * R : (kt + 1) * R],
                                rhs=wt[:, kt * FH + s * 512 : kt * FH + (s + 1) * 512],
                                start=(kt == 0),
                                stop=(kt == 5),
                            )
                        nc.scalar.copy(
                            out=he_tab[:, hf * FH + s * 512 : hf * FH + (s + 1) * 512],
                            in_=pst[:R, :],
                        )

                att_t = sp.tile([128, HD], BF16, tag="att")
                nc.sync.dma_start(out=att_t[:], in_=att_in[l])
                bias_b = sp.tile([128, D], F32, tag="biasb")
                nc.sync.dma_start(out=bias_b[:], in_=bias_in[l])

                prep_half(0)
                prep_half(1)
                ws_next = (
                    [load_w_half(ws_in, l + 1, hf) for hf in range(2)]
                    if (do_edge and l + 1 < n_layers)
                    else None
                )

                # ---- two head-pair sweeps ----
                for half in range(2 if do_edge else 0):
                    hc = half * FH

                    def attn_logits(z, logit, p_t, p_b):
                        """logits -> p (no value folding)."""
                        for hh in range(2):
                            hg = half * 2 + hh
                            nc.vector.scalar_tensor_tensor(
                                out=z[:, hh * D : (hh + 1) * D],
                                in0=z[:, hh * D : (hh + 1) * D],
                                scalar=1.0,
                                in1=att_t[:, hg * D : (hg + 1) * D],
                                op0=ALU.mult,
                                op1=ALU.mult,
                                accum_out=logit[:, hh : hh + 1],
                            )
                        nc.scalar.activation(out=p_t[:], in_=logit[:], func=AF.Exp)
                        nc.vector.tensor_copy(out=p_b[:], in_=p_t[:])

                    def make_ph(oh, p_t):
                        phs = []
                        for hh in range(2):
                            ph = sp.tile([128, 128], BF16, tag="ph", bufs=12)
                            nc.vector.tensor_scalar_mul(
                                ph[:], in0=oh, scalar1=p_t[:, hh : hh + 1]
                            )
                            phs.append(ph)
                        return phs

                    pending = []

                    def emit_interleave(w2, g2):
                        """Next-layer transpose + hs projection + staging for
                        window w2 (deferred one window so it back-fills PE)."""
                        rows2 = slice(w2 * 128, (w2 + 1) * 128)
                        for kt in range(6):
                            pt = pp.tile([128, 128], F32, tag="pa", bufs=2)
                            nc.tensor.transpose(
                                out=pt[:],
                                in_=g2[:, kt * 128 : (kt + 1) * 128],
                                identity=ident[:],
                            )
                            nc.scalar.copy(
                                out=xT[
                                    :, kt * NL + w2 * 128 : kt * NL + (w2 + 1) * 128
                                ],
                                in_=pt[:],
                            )
                        if ws_next is None:
                            return
                        for hf in range(2):
                            ev = sp.tile([128, FH], BF16, tag="ev", bufs=2)
                            for s in range(3):
                                ps = pp.tile([128, 512], F32, tag="pu", bufs=3)
                                for kt in range(6):
                                    nc.tensor.matmul(
                                        ps[:],
                                        lhsT=xT[
                                            :,
                                            kt * NL + w2 * 128 : kt * NL
                                            + (w2 + 1) * 128,
                                        ],
                                        rhs=ws_next[hf][
                                            :,
                                            kt * FH + s * 512 : kt * FH + (s + 1) * 512,
                                        ],
                                        start=(kt == 0),
                                        stop=(kt == 5),
                                    )
                                nc.scalar.copy(
                                    out=ev[:, s * 512 : (s + 1) * 512], in_=ps[:]
                                )
                            nc.sync.dma_start(out=ag_nxt[hf][rows2, :], in_=ev[:])
                        for jc in stage_sched[w2]:
                            stage_chunk(jc, ag_nxt)

                    # self-loop pre-pass over all windows: needs no exchanged
                    # rows, so it fills the pipeline while the AllToAll for
                    # this half is still in flight.
                    selfp = []
                    for w in range(NW):
                        rows = slice(w * 128, (w + 1) * 128)
                        hdw0 = sp.tile([128, FH], BF16, tag="hdw", bufs=2)
                        nc.sync.dma_start(out=hdw0[:], in_=hd_own[rows, hc : hc + FH])
                        hsw0 = sp.tile([128, FH], BF16, tag="hsw", bufs=2)
                        nc.sync.dma_start(out=hsw0[:], in_=ag_cur[half][rows, :])
                        z2 = sp.tile([128, FH], BF16, tag="z", bufs=2)
                        for s in range(3):
                            sc = slice(s * 512, (s + 1) * 512)
                            pu2 = pp.tile([128, 512], F32, tag="pu", bufs=3)
                            nc.tensor.matmul(
                                pu2[:],
                                lhsT=cdt[:, rows],
                                rhs=he_tab[:, hc + s * 512 : hc + (s + 1) * 512],
                                start=True,
                                stop=False,
                            )
                            nc.tensor.matmul(
                                pu2[:], lhsT=identb[:], rhs=hsw0[:, sc],
                                start=False, stop=False,
                            )
                            nc.tensor.matmul(
                                pu2[:], lhsT=identb[:], rhs=hdw0[:, sc],
                                start=False, stop=True,
                            )
                            nc.scalar.activation(
                                out=z2[:, sc], in_=pu2[:], func=AF.Prelu,
                                alpha=NEG_SLOPE,
                            )
                        logit2 = sp.tile([128, 2], F32, tag="logit", bufs=4)
                        p2 = sp.tile([128, 2], F32, tag="pts", bufs=NW + 2)
                        p2b = sp.tile([128, 2], BF16, tag="ptbs", bufs=NW + 2)
                        attn_logits(z2, logit2, p2, p2b)
                        selfp.append((p2, p2b))

                    ci = 0
                    for w in range(NW):
                        rows = slice(w * 128, (w + 1) * 128)
                        po = pp.tile([128, FH], F32, tag="po")
                        pd = pp.tile([128, 2], F32, tag="pa", bufs=2)
                        hdw = sp.tile([128, FH], BF16, tag="hdw", bufs=2)
                        nc.sync.dma_start(out=hdw[:], in_=hd_own[rows, hc : hc + FH])
                        hsw = sp.tile([128, FH], BF16, tag="hsw", bufs=2)
                        nc.sync.dma_start(out=hsw[:], in_=ag_cur[half][rows, :])
                        nreal = cpw[w]
                        vals = []  # (value rows tile, one-hot lhsT AP, p tile)

                        p2, p2b = selfp[w]
                        vals.append((hsw, make_ph(identb[:], p2)))

                        for c in range(nreal):
                            cidx = ci + c
                            ccols = slice(cidx * 128, (cidx + 1) * 128)
                            hsg = sp.tile([128, FH], BF16, tag="hsg", bufs=8)
                            icol = half * max(nch, 1) + cidx
                            nc.gpsimd.indirect_dma_start(
                                out=hsg[:],
                                out_offset=None,
                                in_=(rcv if do_cc else snd)[:, :],
                                in_offset=bass.IndirectOffsetOnAxis(
                                    ap=idx_t[:, icol : icol + 1], axis=0
                                ),
                            )
                            z = sp.tile([128, FH], BF16, tag="z", bufs=2)
                            for s in range(3):
                                sc = slice(s * 512, (s + 1) * 512)
                                pu = pp.tile([128, 512], F32, tag="pu", bufs=3)
                                nc.tensor.matmul(
                                    pu[:],
                                    lhsT=ohd_t[:, ccols],
                                    rhs=hdw[:, sc],
                                    start=True,
                                    stop=False,
                                )
                                nc.tensor.matmul(
                                    pu[:],
                                    lhsT=oha_t[:, ccols],
                                    rhs=he_tab[:, hc + s * 512 : hc + (s + 1) * 512],
                                    start=False,
                                    stop=False,
                                )
                                nc.tensor.matmul(
                                    pu[:],
                                    lhsT=identb[:],
                                    rhs=hsg[:, sc],
                                    start=False,
                                    stop=True,
                                )
                                nc.scalar.activation(
                                    out=z[:, sc], in_=pu[:], func=AF.Prelu,
                                    alpha=NEG_SLOPE,
                                )
                            logit = sp.tile([128, 2], F32, tag="logit", bufs=4)
                            p_t = sp.tile([128, 2], F32, tag="pt", bufs=4)
                            p_b = sp.tile([128, 2], BF16, tag="ptb", bufs=4)
                            attn_logits(z, logit, p_t, p_b)
                            vals.append((hsg, make_ph(eq_t[:, ccols], p_t)))

                        # scatter: po += ph_h^T @ hs per head (bank-contained
                        # segments; sequential groups within shared banks)
                        nv = len(vals)
                        for hh, a, b in (
                            (0, 0, 512), (0, 512, 768),
                            (1, 768, 1024), (1, 1024, 1536),
                        ):
                            for i, (vt, phs) in enumerate(vals):
                                nc.tensor.matmul(
                                    po[:, a:b],
                                    lhsT=phs[hh][:],
                                    rhs=vt[:, a:b],
                                    start=(i == 0),
                                    stop=(i == nv - 1),
                                )
                        for hh in range(2):
                            for i, (vt, phs) in enumerate(vals):
                                nc.tensor.matmul(
                                    pd[:, hh : hh + 1],
                                    lhsT=phs[hh][:],
                                    rhs=ones_b[:],
                                    start=(i == 0),
                                    stop=(i == nv - 1),
                                )

                        # finalize window for this head pair
                        rden = sp.tile([128, 2], F32, tag="rden", bufs=2)
                        nc.vector.reciprocal(out=rden[:], in_=pd[:])
                        nc.vector.tensor_scalar_mul(
                            rden[:], in0=rden[:], scalar1=1.0 / H
                        )
                        if half == 0:
                            acct = sp.tile([128, D], BF16, tag="acct", bufs=2)
                            nc.scalar.activation(
                                out=acct[:],
                                in_=po[:, 0:D],
                                func=AF.Copy,
                                scale=rden[:, 0:1],
                            )
                            nc.vector.scalar_tensor_tensor(
                                out=acct[:],
                                in0=po[:, D : 2 * D],
                                scalar=rden[:, 1:2],
                                in1=acct[:],
                                op0=ALU.mult,
                                op1=ALU.add,
                            )
                            nc.sync.dma_start(out=acc_d[rows, :], in_=acct[:])
                        else:
                            acct = sp.tile([128, D], BF16, tag="acct", bufs=2)
                            nc.sync.dma_start(out=acct[:], in_=acc_d[rows, :])
                            fin = sp.tile([128, D], F32, tag="fin", bufs=2)
                            nc.vector.scalar_tensor_tensor(
                                out=fin[:],
                                in0=po[:, 0:D],
                                scalar=rden[:, 0:1],
                                in1=acct[:],
                                op0=ALU.mult,
                                op1=ALU.add,
                            )
                            nc.vector.scalar_tensor_tensor(
                                out=fin[:],
                                in0=po[:, D : 2 * D],
                                scalar=rden[:, 1:2],
                                in1=fin[:],
                                op0=ALU.mult,
                                op1=ALU.add,
                            )
                            nc.vector.tensor_add(out=fin[:], in0=fin[:], in1=bias_b[:])
                            g_t = sp.tile([128, D], F32, tag="fin2", bufs=3)
                            nc.scalar.activation(
                                out=g_t[:], in_=fin[:], func=AF.Gelu_apprx_tanh
                            )
                            xc = sp.tile([128, D], F32, tag="fin", bufs=2)
                            nc.sync.dma_start(out=xc[:], in_=x_cur[rows, :])
                            nc.vector.tensor_add(out=g_t[:], in0=g_t[:], in1=xc[:])
                            nc.sync.dma_start(out=x_next[rows, :], in_=g_t[:])
                            pending.append((w, g_t))
                        ci += nreal
                        if len(pending) > 1:
                            emit_interleave(*pending.pop(0))
                    while pending:
                        emit_interleave(*pending.pop(0))

                if do_edge:
                    if ws_next is not None:
                        exchange()
                    x_cur = x_next

            # ================= projection + LayerNorm + gelu =================
            if os.environ.get("GAT_PROJ", "1") != "1":
                for m in range(NW):
                    rows = slice(m * 128, (m + 1) * 128)
                    ct = sp.tile([128, D], F32, tag="fin", bufs=2)
                    nc.sync.dma_start(out=ct[:], in_=x_cur[rows, :])
                    nc.sync.dma_start(out=out_t[rows, :], in_=ct[:])
            else:
                # proj weights, laid out [128, kt*768+f]; shares tag "W"
                pwt = sp.tile([128, 6 * D], BF16, tag="W", bufs=2)
                for kt in range(6):
                    nc.sync.dma_start(
                        out=pwt[:, kt * D : (kt + 1) * D],
                        in_=pw_in[kt * 128 : (kt + 1) * 128, :],
                    )
                # xT already holds the final x (interleaved transposes)
                for m in range(NW):
                    rows = slice(m * 128, (m + 1) * 128)
                    ps = pp.tile([128, D], F32, tag="po")
                    for kt in range(6):
                        for a, b in ((0, 512), (512, 768)):
                            nc.tensor.matmul(
                                ps[:, a:b],
                                lhsT=xT[:, kt * NL + m * 128 : kt * NL + (m + 1) * 128],
                                rhs=pwt[:, kt * D + a : kt * D + b],
                                start=(kt == 0),
                                stop=(kt == 5),
                            )
                    y0 = sp.tile([128, D], F32, tag="fin", bufs=2)
                    nc.vector.tensor_add(out=y0[:], in0=ps[:], in1=pbb[:])
                    mu = sp.tile([128, 1], F32, tag="stats", bufs=4)
                    nc.vector.tensor_reduce(
                        out=mu[:], in_=y0[:], axis=mybir.AxisListType.X, op=ALU.add
                    )
                    nc.vector.tensor_scalar_mul(mu[:], in0=mu[:], scalar1=1.0 / D)
                    xc2 = sp.tile([128, D], F32, tag="fin2", bufs=3)
                    nc.vector.tensor_scalar_sub(xc2[:], in0=y0[:], scalar1=mu[:])
                    var = sp.tile([128, 1], F32, tag="stats", bufs=4)
                    nc.vector.scalar_tensor_tensor(
                        out=y0[:],
                        in0=xc2[:],
                        scalar=1.0,
                        in1=xc2[:],
                        op0=ALU.mult,
                        op1=ALU.mult,
                        accum_out=var[:],
                    )
                    nc.vector.tensor_scalar(
                        var[:],
                        in0=var[:],
                        scalar1=1.0 / D,
                        scalar2=LN_EPS,
                        op0=ALU.mult,
                        op1=ALU.add,
                    )
                    sd = sp.tile([128, 1], F32, tag="stats", bufs=4)
                    nc.scalar.activation(out=sd[:], in_=var[:], func=AF.Sqrt)
                    rstd = sp.tile([128, 1], F32, tag="stats", bufs=4)
                    nc.vector.reciprocal(out=rstd[:], in_=sd[:])
                    nc.vector.tensor_scalar_mul(y0[:], in0=xc2[:], scalar1=rstd[:])
                    nc.vector.tensor_mul(out=y0[:], in0=y0[:], in1=lngb[:])
                    nc.vector.tensor_add(out=y0[:], in0=y0[:], in1=lnbb[:])
                    og = sp.tile([128, D], F32, tag="fin", bufs=2)
                    nc.scalar.activation(out=og[:], in_=y0[:], func=AF.Gelu_apprx_tanh)
                    nc.sync.dma_start(out=out_t[rows, :], in_=og[:])

    _split_multi_waits(nc)
    return nc


# ----------------------------------------------------------------------------
# Host side
# ----------------------------------------------------------------------------


def _preprocess(edge_index, edge_attr):
    src = np.asarray(edge_index[0], dtype=np.int64)
    dst = np.asarray(edge_index[1], dtype=np.int64)
    attr = np.asarray(edge_attr, dtype=np.int64)
    deg = np.bincount(dst, minlength=N).astype(np.int64)

    # ---- node permutation: balance cores by degree, pack windows to EPW ----
    order = np.argsort(-deg, kind="stable")
    core_of = np.empty(N, np.int64)
    cload = np.zeros(NCORES, np.int64)
    ccnt = np.zeros(NCORES, np.int64)
    for n in order:
        k = int(np.argmin(np.where(ccnt < NL, cload, np.iinfo(np.int64).max)))
        core_of[n] = k
        cload[k] += deg[n]
        ccnt[k] += 1

    new_id = np.empty(N, np.int64)
    for k in range(NCORES):
        nodes = np.where(core_of == k)[0]
        nodes = nodes[np.argsort(-deg[nodes], kind="stable")]
        wload = np.zeros(NW, np.int64)
        wcnt = np.zeros(NW, np.int64)
        wassign = np.empty(len(nodes), np.int64)
        for i, n in enumerate(nodes):
            d = deg[n]
            open_w = wcnt < WSZ
            fits = open_w & (wload + d <= EPW)
            if fits.any():
                # best-fit: fullest window that still fits
                j = int(np.argmax(np.where(fits, wload, -1)))
            else:
                j = int(np.argmin(np.where(open_w, wload, np.iinfo(np.int64).max)))
            wassign[i] = j
            wload[j] += d
            wcnt[j] += 1
        # swap-rebalance: no window should exceed EPW (each extra 128 costs a
        # whole padded chunk on every core via the max-over-cores cpw)
        degs = deg[nodes]
        for _ in range(200):
            wo = int(np.argmax(wload))
            if wload[wo] <= EPW:
                break
            done = False
            cand_a = np.where(wassign == wo)[0]
            cand_a = cand_a[np.argsort(-degs[cand_a], kind="stable")]
            for wu in np.argsort(wload, kind="stable"):
                if done or wload[wu] >= wload[wo]:
                    break
                cand_b = np.where(wassign == wu)[0]
                b = int(cand_b[np.argmin(degs[cand_b])])
                for a in cand_a:
                    da, db = int(degs[a]), int(degs[b])
                    if da > db and wload[wu] + da - db <= EPW:
                        wassign[a], wassign[b] = wu, wo
                        wload[wo] += db - da
                        wload[wu] += da - db
                        done = True
                        break
            if not done:
                break
        # relabel windows by descending load so full windows align across cores
        worder = np.argsort(-wload, kind="stable")
        wrank = np.empty(NW, np.int64)
        wrank[worder] = np.arange(NW)
        slot = np.zeros(NW, np.int64)
        for i, n in enumerate(nodes):
            j = wrank[wassign[i]]
            new_id[n] = k * NL + j * 128 + slot[j]
            slot[j] += 1

    perm = np.empty(N, np.int64)  # new -> old
    perm[new_id] = np.arange(N)

    srcN = new_id[src]
    dstN = new_id[dst]
    k_e = dstN // NL
    w_e = (dstN % NL) // 128
    slot_e = dstN % 128

    # real chunks per window index (max over cores)
    loads = np.zeros((NCORES, NW), np.int64)
    np.add.at(loads, (k_e, w_e), 1)
    cpw = [int(x) for x in np.ceil(loads.max(axis=0) / 128).astype(np.int64)]
    nch = int(sum(cpw))
    cstart = np.concatenate([[0], np.cumsum(cpw)])

    # ---- A2A request lists ----
    own_e = srcN // NL  # owner core of each edge's src
    reqs = [[None] * NCORES for _ in range(NCORES)]  # reqs[j][o]
    pmax = 1
    for j in range(NCORES):
        em = k_e == j
        for o in range(NCORES):
            rows = np.unique(srcN[em & (own_e == o)] % NL)
            reqs[j][o] = rows
            pmax = max(pmax, len(rows))
    pc = -(-pmax // 128)
    ppad = pc * 128

    sreq_all = []
    for k in range(NCORES):
        arr = np.zeros((128, NCORES * pc), np.int32)
        for j in range(NCORES):
            r = reqs[j][k]
            col = np.zeros(ppad, np.int32)
            col[: len(r)] = r
            arr[:, j * pc : (j + 1) * pc] = col.reshape(pc, 128).T
        sreq_all.append(np.ascontiguousarray(arr))

    # earliest sweep-B window after which each staging chunk can be gathered
    # (requests are sorted, so chunk cc covers a contiguous row range)
    wmax = np.zeros(NCORES * pc, np.int64)
    for j in range(NCORES):
        for k in range(NCORES):
            r = reqs[j][k]
            for cc in range(pc):
                seg = r[cc * 128 : (cc + 1) * 128]
                if len(seg):
                    wmax[j * pc + cc] = max(wmax[j * pc + cc], int(seg.max()) // 128)
    stage_sched = [[] for _ in range(NW)]
    for jc in range(NCORES * pc):
        stage_sched[int(wmax[jc])].append(jc)

    # ---- per-core chunk data ----
    # idx columns [0:nch] = half-A receive rows, [nch:2nch] = half-B
    # (halves row-interleaved per source block: A at +0, B at +ppad)
    idx_all, ohd_all, oha_all, eq_all = [], [], [], []
    for j in range(NCORES):
        idx = np.zeros((128, 2 * nch), np.int32)
        ohd = np.zeros((128, nch * 128), np.float32)
        oha = np.zeros((R, nch * 128), np.float32)
        eqm = np.zeros((128, nch * 128), np.float32)
        em = np.where(k_e == j)[0]
        eo = em[np.argsort(w_e[em], kind="stable")]
        wcnt2 = np.bincount(w_e[em], minlength=NW)
        wst = np.concatenate([[0], np.cumsum(wcnt2)])
        for w in range(NW):
            es = eo[wst[w] : wst[w + 1]]
            base = cstart[w] * 128
            for i, e in enumerate(es):
                c, r2 = divmod(i, 128)
                col = base + c * 128 + r2
                o = own_e[e]
                pos = int(np.searchsorted(reqs[j][o], srcN[e] % NL))
                idx[r2, cstart[w] + c] = o * 2 * ppad + pos
                idx[r2, nch + cstart[w] + c] = o * 2 * ppad + ppad + pos
                ohd[slot_e[e], col] = 1.0
                oha[attr[e], col] = 1.0
                eqm[r2, base + c * 128 + slot_e[e]] = 1.0
        idx_all.append(idx)
        ohd_all.append(ohd.astype(ml_dtypes.bfloat16))
        oha_all.append(oha.astype(ml_dtypes.bfloat16))
        eq_all.append(eqm.astype(ml_dtypes.bfloat16))

    # ---- Cdiv (self-loop mean edge attr), new node order ----
    C = np.zeros((N, R), np.float32)
    np.add.at(C, (dstN, attr), 1.0)
    degN = np.bincount(dstN, minlength=N).astype(np.float32)
    Cdiv = C / np.maximum(degN, 1.0)[:, None]

    return (
        cpw, nch, pc, stage_sched, perm, new_id,
        sreq_all, idx_all, ohd_all, oha_all, eq_all, Cdiv,
    )


_cache = {}
_prep_cache = {}
LAST_RESULTS = None
LAST_EXEC_NS = None


def prepare(**inputs):
    x = np.asarray(inputs["x"], np.float32)
    rel_emb = np.asarray(inputs["rel_emb"], np.float32)
    w_src = np.asarray(inputs["w_src"], np.float32)
    w_dst = np.asarray(inputs["w_dst"], np.float32)
    w_edge = np.asarray(inputs["w_edge"], np.float32)
    att = np.asarray(inputs["att"], np.float32)
    bias = np.asarray(inputs["bias"], np.float32)
    proj_w = np.asarray(inputs["proj_w"], np.float32)
    proj_b = np.asarray(inputs["proj_b"], np.float32)
    ln_g = np.asarray(inputs["ln_g"], np.float32)
    ln_b = np.asarray(inputs["ln_b"], np.float32)
    edge_index = np.asarray(inputs["edge_index"], np.int32)
    edge_attr = np.asarray(inputs["edge_attr"], np.int32)

    ekey = (edge_index.tobytes(), edge_attr.tobytes())
    ck = hash(ekey)
    if ck not in _prep_cache:
        _prep_cache[ck] = _preprocess(edge_index, edge_attr)
    (
        cpw, nch, pc, stage_sched, perm, new_id,
        sreq_all, idx_all, ohd_all, oha_all, eq_all, Cdiv,
    ) = _prep_cache[ck]

    key = (tuple(cpw), nch, pc, tuple(tuple(s) for s in stage_sched))
    if key not in _cache:
        _cache[key] = build_program(cpw, nch, pc, stage_sched)
    nc = _cache[key]

    bf = lambda a: np.ascontiguousarray(a).astype(ml_dtypes.bfloat16)
    ws_b = bf(w_src.reshape(L, D, HD))
    wd_b = bf(w_dst.reshape(L, D, HD))
    we_b = bf(w_edge.reshape(L, D, HD))
    rel_b = bf(rel_emb)
    att_rep = bf(np.broadcast_to(att.reshape(L, 1, HD), (L, 128, HD)))
    bias_rep = np.ascontiguousarray(
        np.broadcast_to(bias.reshape(L, 1, D), (L, 128, D)), dtype=np.float32
    )
    pw_b = bf(proj_w)
    pb_rep = bf(np.broadcast_to(proj_b, (128, D)))
    lng_rep = bf(np.broadcast_to(ln_g, (128, D)))
    lnb_rep = bf(np.broadcast_to(ln_b, (128, D)))

    in_maps = []
    for k in range(NCORES):
        rows = perm[k * NL : (k + 1) * NL]
        in_maps.append(
            {
                "x_own": np.ascontiguousarray(x[rows]),
                "idx": idx_all[k],
                "sreq": sreq_all[k],
                "ohd": ohd_all[k],
                "oha": oha_all[k],
                "eq": eq_all[k],
                "cdivT": bf(Cdiv[k * NL : (k + 1) * NL].T),
                "w_src": ws_b,
                "w_dst": wd_b,
                "w_edge": we_b,
                "rel_emb": rel_b,
                "att_rep": att_rep,
                "bias_rep": bias_rep,
                "proj_w": pw_b,
                "pb_rep": pb_rep,
                "lng_rep": lng_rep,
                "lnb_rep": lnb_rep,
            }
        )
    return nc, in_maps, new_id


def kernel(**inputs):
    nc, in_maps, new_id = prepare(**inputs)
    trace = os.environ.get("GAT_TRACE", "0") == "1"
    res = run_bass_kernel_spmd(nc, in_maps, core_ids=list(range(NCORES)), trace=trace)
    global LAST_RESULTS, LAST_EXEC_NS
    LAST_RESULTS = res.results
    LAST_EXEC_NS = res.exec_time_ns
    arr = np.concatenate([res.results[k]["out"] for k in range(NCORES)], axis=0)
    return arr[new_id].astype(np.float32)
